# revision 11
# baseline (speedup 1.0000x reference)
"""Trainium2 Bass kernel for nn_Attention (dense transformer spatial attention).

Reference computation (per batch b of 4):
  X = x[b] reshaped [256, 4096]                      (4096 = 64*64 pixels)
  QKV = w_qkv @ X -> [384, 4096]; q,k,v = split(QKV) each [128, 4096]
  per head h (4 heads x 32 dims): sim = (q_h*scale)^T k_h   [4096, 4096]
  attn = softmax(sim, axis=-1); out_h = attn @ v_h^T        [4096, 32]
  H = concat_heads -> [128, 4096]; out = w_out @ H + b_out  [256, 4096]

Sharding: 8 cores = (batch b in 0..3) x (query half qh in 0..1).
Each core gets full X_b (for K/V) plus its query-half slice, computes
attention output for its 2048 queries over all 4096 keys, and the final
projection.  X columns are permuted per core: [own query half | other half],
so q projections read a contiguous slice and j-order is core-local (softmax
is permutation invariant over keys).

Design notes (tuned against the TimelineSim cost model; steady state is
DVE-exp-paced at ~1.26us/jt-step, wall ~189us):
  - blob256 (weights + permuted x) is BF16: halves input DMA bytes and
    makes every projection a bf16 matmul (always 1 PE cycle/row; an f32r
    moving operand under 256 output columns would run at 4 cycles/row).
    Input pieces are merged (one DMA per column range covering both
    128-row halves via a 3-dim access pattern) to reduce serialization on
    the single-slot HWDGE (~625ns per DMA), and the first piece carries
    only the weights so the first projections start ~1us earlier.
  - sim is computed TRANSPOSED (simT[j,i]) via K=32 row-packed bf16
    matmuls (tile_position=(32h,0)).  THREE rotating [128,1024] PSUM
    buffers hold sim tiles (6 banks); with the 2 AV-accumulator banks
    PSUM is exactly full, which rules out wider exp instructions (2048-col
    tiles would need 4 slots = 10 banks).
  - softmax exp is SPLIT between ScalarE (true exp activation, scale
    folded) and the DVE (Schraudolph approximate exp: q is pre-scaled on
    the host by SCALE*128/ln2, so exp bf16 BITS = int16(sim + B) -- one
    tensor_scalar add with fp32->int16 convert writes bf16-bit-pattern
    output directly; ~3% weight error, cancels further in the softmax
    ratio).  DVE always takes group 0 so each rotating sim buffer
    alternates consumer engines.  SPLIT_STEPS hands ScalarE most of the
    DVE tile on the first step of chunks 1-3 so the DVE can absorb the
    previous chunk's epilogue burst (reciprocal/normalize/copies) without
    idling.  Donating columns on OTHER steps always loses: a sub-tile
    donation adds a ~185ns ScalarE instruction overhead and ties both
    engines to one PSUM tile, stalling the 3-slot rotation.
  - vT is projected DIRECTLY as x^T @ wvT (lhsT = the resident x tile,
    K=256): no v tensor, no PE transposes, no extra evacuation -- saves
    ~4096 vector-engine columns vs the transpose pipeline and avoids the
    DMA-xbar transpose races.  The softmax denominators come from an N=1
    ones-column matmul per AV accumulator (~1 PE cycle each).
  - AV is FLIPPED vs the naive layout: stationary = exp tile [128j x
    128i], moving = vT [128j, 32].  16 accumulators [128,33] at stride-64
    slots share 2 PSUM banks; only the first matmul touching each bank
    uses start=True.  AV emission lags the exp stream (LAG_START after a
    chunk boundary, LAG_MID mid-chunk, 1 for the last steps of the final
    chunk) so accumulator-bank WAW never parks at the head of the
    in-order PE queue; the lag values were swept against TimelineSim.
  - The flipped AV emits h as [i, hd]; per 128-query block it is
    normalized (reciprocal of the ones column, free-dim broadcast
    multiply), PE-transposed back to [hd, i] (f32r) and projected with
    full-width woutT in one N=256 f32r matmul.  The bias is pre-loaded
    into the projection PSUM by a K=1 ones-row matmul so the final
    evacuation is a plain copy on whichever engine is idler.  Epilogue
    transposes/projections reuse consumed avt regions; the last chunk
    routes projections through the then-idle sim buffers instead.
  - Phase 1 evacuations alternate ScalarE/DVE under the input-DMA shadow
    (q first-piece on DVE so k and q evacuate in parallel); identity
    transposes pre-warm the PE p-state; the second x-half's K and vT
    production is interleaved into chunk 0 of the main loop through the
    same rotating sim buffers.
  Measured dead ends (cost model): fp8 q/k with DoubleRow matmuls halves
  sim PE time but adds ~4.5%% output error (gate 2%%); fp8 exp tiles
  either NaN on >5.5-sigma sims (e4m3) or add ~2.5%% noise (e5m2); GpSimd
  cannot touch PSUM; DMA cannot read PSUM; DVE 2x/4x modes need all-SBUF
  or 2-byte inputs, and sim is forced f32-in-PSUM.
"""

import math
import os

import ml_dtypes
import numpy as np

def _k(name, default):
    return int(os.environ.get(name, default))

import concourse.bacc as bacc
import concourse.bass as bass
import concourse.masks as masks
import concourse.mybir as mybir
import concourse.tile as tile
from concourse.bass_utils import run_bass_kernel_spmd

F32 = mybir.dt.float32
F32R = mybir.dt.float32r
BF16 = mybir.dt.bfloat16
I16 = mybir.dt.int16

HEADS = 4
DH = 32                      # dim per head
C = 256                      # input channels
NJ = 4096                    # keys per batch (64*64)
NI = 2048                    # queries per core (half of 4096)
JT = 128                     # j tile (partition dim of simT)
NJT = NJ // JT               # 32 j tiles
CHUNK = 512                  # i chunk held in AV psum accumulators
NCHUNK = NI // CHUNK         # 4
NIB = CHUNK // 128           # 4 i-blocks per chunk
SCALE = float(DH) ** -0.5
LN2 = math.log(2.0)
# q is pre-scaled by SCALE * 128/ln2 on the host; ScalarE exp then uses
# scale=ln2/128, and the DVE Schraudolph path just adds SCHRAU_B and
# converts to int16 (the bf16 bit pattern of exp).
QPRE = 128.0 / LN2
SCHRAU_C = 0.05              # Schraudolph correction (centers rel err ~+-3%)
SCHRAU_B = 128.0 * (127.0 - SCHRAU_C) + 0.5   # +0.5: f32->i16 truncates

XW = 384 + NJ                # blob256 width: [wq|wk|wvT (384) | x perm (4096)]

# (chunk, jt) -> columns of the DVE's group-0 tile that ScalarE takes
# instead (a partial hole: smooth rebalancing of ScalarE ~ DVE busy time,
# and air for the DVE's epilogue burst at chunk starts).
SPLIT_COLS = _k("SPLIT_COLS", 960)
SPLIT_MODE = _k("SPLIT_MODE", 0)
if SPLIT_MODE == 0:
    SPLIT_STEPS = {(c, 0): SPLIT_COLS for c in (1, 2, 3)}
elif SPLIT_MODE == 1:
    SPLIT_STEPS = {(c, 1): 1024 for c in (1, 2, 3)}
elif SPLIT_MODE == 2:
    SPLIT_STEPS = {(c, 0): 1024 for c in (1, 2, 3)}
elif SPLIT_MODE == 3:
    SPLIT_STEPS = {(c, 0): 960 for c in (1, 2, 3)}
    SPLIT_STEPS.update({(c, 2): 512 for c in (1, 2, 3)})
elif SPLIT_MODE == 4:
    SPLIT_STEPS = {(c, 0): 960 for c in (1, 2, 3)}
    SPLIT_STEPS[(0, 1)] = 512
elif SPLIT_MODE == 5:
    SPLIT_STEPS = {}
_SE = _k("SPLIT_EVERY", 0)
if _SE:
    for _c in range(4):
        for _jt in range(4, 32, 2):
            SPLIT_STEPS.setdefault((_c, _jt), _SE)
LAG_START = _k("LAG_START", 7)
LAG_C0 = _k("LAG_C0", 2)
LAG_MID = _k("LAG_MID", 2)
LAG_SW = _k("LAG_SW", 12)
TAIL_JT = _k("TAIL_JT", 29)
TAPER_ALL = _k("TAPER_ALL", 0)
FINE_PRO = _k("FINE_PRO", 2)
MERGE_DMA = _k("MERGE_DMA", 1)
EMIT_ORDER = _k("EMIT_ORDER", 0)
AV_SPLIT = _k("AV_SPLIT", 0)
SPLIT_EVERY = _k("SPLIT_EVERY", 0)
P1B_MODE = _k("P1B_MODE", 0)
TAIL_POOL = _k("TAIL_POOL", 5)
PRO_POOL = _k("PRO_POOL", 0)
WARMUP_N = _k("WARMUP_N", 10)


def build_kernel(dbg=False):
    nc = bacc.Bacc("TRN2", debug=False, num_devices=8)

    blob256_d = nc.dram_tensor("blob256", [C, XW], BF16, kind="ExternalInput").ap()
    blob128_d = nc.dram_tensor("blob128", [128, 2 * C], F32R, kind="ExternalInput").ap()
    out_d = nc.dram_tensor("out_t", [NI, C], F32, kind="ExternalOutput").ap()
    if dbg:
        dumps = {n: nc.dram_tensor("dump_" + n, s, d, kind="ExternalOutput").ap()
                 for n, s, d in [
                     ("q", [128, NI], F32), ("k", [128, NJ], F32),
                     ("vT", [128, NJT * 128], BF16),
                     ("rec", [128, NCHUNK * 16], F32),
                     ("ex0", [128, 2048], BF16)]}

    with tile.TileContext(nc) as tc:
        with (
            tc.tile_pool(name="singles", bufs=1) as singles,
            tc.tile_pool(name="expp", bufs=_k("EXPP", 32)) as expp,
            tc.tile_pool(name="hp", bufs=_k("HP", 3)) as hp,
            tc.tile_pool(name="htp", bufs=_k("HTP", 3)) as htp,
            tc.tile_pool(name="otp", bufs=_k("OTP", 4)) as otp,
            tc.tile_pool(name="recp", bufs=_k("RECP", 2)) as recp,
            tc.tile_pool(name="psim", bufs=1, space="PSUM") as psim,
            tc.tile_pool(name="pav", bufs=1, space="PSUM") as pav,
        ):
            # ---- resident SBUF tensors ----
            blob_sb = singles.tile([128, 2, XW], BF16)
            wq_sb = blob_sb[:, :, 0:128]
            wk_sb = blob_sb[:, :, 128:256]
            wvT_sb = blob_sb[:, :, 256:384]
            x_sb = blob_sb[:, :, 384:XW]
            b128_sb = singles.tile([128, 2 * C], F32R)
            woutT_sb = b128_sb[:, 0:C]
            bias_sb = b128_sb[:, C:2 * C]
            q_sb = singles.tile([128, NI], F32R)      # rows = 4h x 32d (prescaled)
            k_sb = singles.tile([128, NJ], F32R)
            # vT[j, jt, hd]: projected directly as xT @ wvT (no transpose
            # pipeline); softmax denominators come from a separate ones
            # column via N=1 matmuls
            vT_sb = singles.tile([128, NJT, 128], BF16)
            onesb_sb = singles.tile([128, 1], BF16)
            idr_sb = singles.tile([128, 128], F32R)   # identity for f32r transposes
            ones_sb = singles.tile([1, 128], F32R)    # K=1 bias-broadcast lhsT

            # rotating sim-chain PSUM slot allocator (3 tags x [128,1024])
            SIMTAGS = ("simA", "simB", "simC")
            sim_ctr = [0]

            def sim_tile(shape=(128, 1024), dtype=F32, name="sim"):
                tag = SIMTAGS[sim_ctr[0] % 3]
                sim_ctr[0] += 1
                return psim.tile(list(shape), dtype, tag=tag, name=name)

            # ---- input DMAs (SP engine); x own-query-half first.  The first
            # transfer carries the weights AND the first 512 x columns in one
            # piece (fewer serial DMA latencies before the q projection).
            if MERGE_DMA:
                blob_src = blob256_d.rearrange("(ct p) c -> p ct c", ct=2)
                if FINE_PRO == 3:
                    pieces = ((0, 256), (384, 512), (256, 128), (896, 512),
                              (1408, 512), (1920, 512), (2432, 1024),
                              (3456, 1024))
                elif FINE_PRO == 2:
                    pieces = ((0, 256), (256, 640), (896, 512), (1408, 512),
                              (1920, 512), (2432, 1024), (3456, 1024))
                elif FINE_PRO:
                    pieces = ((0, 384), (384, 512), (896, 512), (1408, 1024),
                              (2432, 2048))
                else:
                    pieces = ((0, 896), (896, 512), (1408, 1024), (2432, 2048))
                for n, (lo, w) in enumerate(pieces):
                    eng = nc.gpsimd if (PRO_POOL and n < 2) else nc.sync
                    eng.dma_start(out=blob_sb[:, :, lo:lo + w],
                                  in_=blob_src[:, :, lo:lo + w])
            else:
                for lo, w in ((0, 896), (896, 512), (1408, 1024), (2432, 2048)):
                    for ct in range(2):
                        nc.sync.dma_start(out=blob_sb[:, ct, lo:lo + w],
                                          in_=blob256_d[ct * 128:(ct + 1) * 128,
                                                        lo:lo + w])
            nc.sync.dma_start(out=b128_sb, in_=blob128_d)

            # identity built once in plain f32 on GpSimd (idle engine), then
            # DVE-converted to the f32r copy the transposes need
            idf_sb = singles.tile([128, 128], F32)
            masks.make_identity(nc, idf_sb)
            nc.vector.tensor_copy(idr_sb, idf_sb)
            nc.vector.memset(onesb_sb, 1.0)
            nc.vector.memset(ones_sb.bitcast(F32), 1.0)

            # trigger the ScalarE exp table load (~2.7us) during phase 1
            warm = singles.tile([1, 1], F32)
            nc.vector.memset(warm, 0.0)
            nc.scalar.activation(warm, warm, mybir.ActivationFunctionType.Exp)

            # warm the PE p-state before the inputs arrive: identity
            # transposes keep the array busy through the cold ramp so the
            # first real projections run at full clock
            pwu = pav.tile([128, 128], F32, tag="av", name="pwu")
            for _ in range(WARMUP_N):
                nc.tensor.transpose(pwu, idf_sb, idf_sb, )

            # ---- phase 1a: q, k/v half 0, vT half 0 ----
            def project(w_slice, x_lo, width, name):
                """[128, width] psum tile = w_slice.T @ x[:, x_lo:x_lo+width]."""
                ps = sim_tile((128, width), F32, name=name)
                for nt0 in range(0, width, 512):
                    w_seg = min(512, width - nt0)
                    for ct in range(2):
                        nc.tensor.matmul(
                            ps[:, nt0:nt0 + w_seg],
                            lhsT=w_slice[:, ct, :],
                            rhs=x_sb[:, ct, x_lo + nt0:x_lo + nt0 + w_seg],
                            start=(ct == 0), stop=(ct == 1),
                        )
                return ps

            # fine-grained first evacs: the first sims need only k cols 0:512
            # and q cols 0:512, so those 512-wide pieces come first and the
            # exp pipeline starts ~2us earlier
            def emit_p1(w_slice, dst, lo, w, dve):
                ps = project(w_slice, lo, w, "ps_p1")
                if dve:
                    nc.vector.tensor_copy(dst[:, lo:lo + w], ps)
                else:
                    nc.scalar.copy(dst[:, lo:lo + w], ps)

            if P1B_MODE == 3:
                p1_list = [
                    (wk_sb, k_sb, 0, 256, False), (wq_sb, q_sb, 0, 512, True),
                    (wk_sb, k_sb, 256, 256, False)]
            elif FINE_PRO:
                p1_list = [
                    (wk_sb, k_sb, 0, 256, False), (wq_sb, q_sb, 0, 512, True),
                    (wk_sb, k_sb, 256, 256, False),
                    (wk_sb, k_sb, 512, 512, False),
                    (wq_sb, q_sb, 512, 512, False),
                    (wq_sb, q_sb, 1024, 1024, False),
                    (wk_sb, k_sb, 1024, 1024, False)]
            else:
                p1_list = [
                    (wk_sb, k_sb, 0, 512, False), (wq_sb, q_sb, 0, 512, False),
                    (wk_sb, k_sb, 512, 512, False), (wq_sb, q_sb, 512, 512, False),
                    (wq_sb, q_sb, 1024, 1024, False),
                    (wk_sb, k_sb, 1024, 1024, False)]
            for w_slice, dst, lo, w, dve in p1_list:
                emit_p1(w_slice, dst, lo, w, dve)

            def emit_vtn(jt0, n, via_sim, dve):
                """vT tiles jt0..jt0+n-1 projected directly from x:
                vt[j, hd] = x[:, j]^T @ wvT (lhsT = x slice, K = 256)."""
                if via_sim:
                    tp = sim_tile((128, n, 128), F32, name="vtp")
                else:
                    tp = pav.tile([128, n, 128], F32, tag="av", name="vtp",
                                  padded_shape=[128, 4, 128])
                for i4 in range(n):
                    jt = jt0 + i4
                    for ct in range(2):
                        nc.tensor.matmul(
                            tp[:, i4, :],
                            lhsT=x_sb[:, ct, jt * 128:(jt + 1) * 128],
                            rhs=wvT_sb[:, ct, :],
                            start=(ct == 0), stop=(ct == 1),
                        )
                if dve:
                    nc.vector.tensor_copy(vT_sb[:, jt0:jt0 + n, :], tp)
                else:
                    nc.scalar.copy(vT_sb[:, jt0:jt0 + n, :], tp)

            def emit_vt4(g, via_sim):
                emit_vtn(4 * g, 4, via_sim, g % 2 == 0)

            for g in range(1 if P1B_MODE == 3 or VT_SIM == 2 else 4):
                emit_vt4(g, via_sim=(VT_SIM == 1))         # vT half 0

            # ---- phase 1b pieces, interleaved into chunk 0 of the main loop,
            # rotating through the same sim-chain psum slots.  Evacuations
            # alternate ScalarE (k) / DVE (vT) to spread the load.
            def emit_k1_piece(n):
                lo = 2048 + n * 1024
                ps = project(wk_sb, lo, 1024, "k1p")
                nc.scalar.copy(k_sb[:, lo:lo + 1024], ps)

            def emit_piece(w_slice, dst, lo, w, dve=False):
                ps = project(w_slice, lo, w, "p1b")
                if dve:
                    nc.vector.tensor_copy(dst[:, lo:lo + w], ps)
                else:
                    nc.scalar.copy(dst[:, lo:lo + w], ps)

            if P1B_MODE == 0:
                PHASE1B = {
                    2: lambda: emit_k1_piece(0),
                    5: lambda: emit_k1_piece(1),
                    8: lambda: emit_vtn(16, 4, True, True),
                    10: lambda: emit_vtn(20, 4, True, False),
                    12: lambda: emit_vtn(24, 4, True, True),
                    14: lambda: emit_vtn(28, 4, True, False),
                }
                if VT_SIM == 2:
                    PHASE1B[1] = lambda: emit_vt4(1, via_sim=True)
                    PHASE1B[3] = lambda: emit_vt4(2, via_sim=True)
                    PHASE1B[6] = lambda: emit_vt4(3, via_sim=True)
            elif P1B_MODE == 1:
                PHASE1B = {
                    6: lambda: emit_k1_piece(0),
                    10: lambda: emit_vtn(16, 4, True, True),
                    13: lambda: emit_k1_piece(1),
                    16: lambda: emit_vtn(20, 4, True, False),
                    20: lambda: emit_vtn(24, 4, True, True),
                    24: lambda: emit_vtn(28, 4, True, False),
                }
            elif P1B_MODE == 2:
                PHASE1B = {
                    4: lambda: emit_k1_piece(0),
                    8: lambda: emit_vtn(16, 4, True, True),
                    10: lambda: emit_k1_piece(1),
                    14: lambda: emit_vtn(20, 4, True, False),
                    18: lambda: emit_vtn(24, 4, True, True),
                    22: lambda: emit_vtn(28, 4, True, False),
                }
            else:
                # need-ordered: each piece lands just before its first use,
                # so early ScalarE/DVE priority goes to the exp stream
                PHASE1B = {
                    1: lambda: emit_piece(wk_sb, k_sb, 512, 512),
                    2: lambda: emit_vt4(1, via_sim=True),
                    3: lambda: emit_piece(wk_sb, k_sb, 1024, 1024),
                    5: lambda: emit_vt4(2, via_sim=True),
                    6: lambda: emit_piece(wq_sb, q_sb, 512, 512),
                    8: lambda: emit_k1_piece(0),
                    9: lambda: emit_vt4(3, via_sim=True),
                    10: lambda: emit_vtn(16, 4, True, True),
                    12: lambda: emit_k1_piece(1),
                    14: lambda: emit_vtn(20, 4, True, False),
                    16: lambda: emit_piece(wq_sb, q_sb, 1024, 1024),
                    18: lambda: emit_vtn(24, 4, True, True),
                    22: lambda: emit_vtn(28, 4, True, False),
                }

            # ---- phase 2: attention main loop ----
            # Each chunk's epilogue is DEFERRED into the next chunk (flushed
            # after its first two sim/exp steps) so the exp engines never
            # starve behind epilogue PE work at chunk boundaries.
            deferred_epi = [None]

            def make_epilogue(avt, c, i0):
                def epi():
                    import contextlib
                    dl = (tc.high_priority(-EPI_DELAY)
                          if EPI_DELAY and c < NCHUNK - 1
                          else contextlib.nullcontext())
                    with dl:
                        _epi_body()

                def _epi_body():
                    rec = recp.tile([128, 16, 1], F32, tag="rec", name="rec")
                    if FINAL2 and c == NCHUNK - 1:
                        # split normalize in halves so the first transposes
                        # start ~0.4us earlier in the tail
                        hsb2 = hp.tile([128, NIB * HEADS, DH], F32R, tag="h",
                                       name="hsb")
                        for hf in range(2):
                            s8 = slice(hf * 8, hf * 8 + 8)
                            nc.vector.reciprocal(out=rec[:, s8, 0],
                                                 in_=avt[:, s8, 32])
                            nc.vector.tensor_tensor(
                                out=hsb2[:, s8, :],
                                in0=avt[:, s8, 0:32],
                                in1=rec[:, s8, :].to_broadcast((128, 8, DH)),
                                op=mybir.AluOpType.mult,
                            )
                        _finish(hsb2)
                        return
                    nc.vector.reciprocal(out=rec[:, :, 0], in_=avt[:, :, 32])
                    if dbg:
                        nc.sync.dma_start(
                            out=dumps["rec"][:, c * 16:(c + 1) * 16],
                            in_=rec[:, :, 0])
                    # one fused normalize for all 16 accumulators (4 separate
                    # TTs pay ~160ns per-op overhead each)
                    hsb = hp.tile([128, NIB * HEADS, DH], F32R, tag="h",
                                  name="hsb")
                    nc.vector.tensor_tensor(
                        out=hsb,
                        in0=avt[:, :, 0:32],
                        in1=rec.to_broadcast((128, NIB * HEADS, DH)),
                        op=mybir.AluOpType.mult,
                    )
                    _finish(hsb)

                def _finish(hsb):
                    hsbs = [hsb[:, ib * HEADS:(ib + 1) * HEADS, :]
                            .rearrange("p h d -> p (h d)") for ib in range(NIB)]
                    # all 4 transposes into bank B (accums 8..15, 512B each)
                    tps = avt[:, 8:16, :].rearrange("p a b -> p (a b)").bitcast(F32R)
                    for ib in range(NIB):
                        nc.tensor.transpose(tps[:, ib * 128:(ib + 1) * 128],
                                            hsbs[ib], idr_sb)
                    htsb = htp.tile([128, 4, 128], F32R, tag="ht", name="htsb")
                    nc.scalar.copy(htsb[:, 0:2, :],
                                   tps[:, 0:256].bitcast(F32).bitcast(F32R))
                    nc.vector.tensor_copy(htsb[:, 2:4, :],
                                          tps[:, 256:512].bitcast(F32))
                    # projections ping-pong through bank A (accums 0..7).
                    # In the final chunk the sim slots are idle and have no
                    # false whole-tile WAR against avt, so projections go there
                    # and run fully parallel instead of serializing behind each
                    # ot read.
                    pjregs = [avt[:, 0:4, :].rearrange("p a b -> p (a b)"),
                              avt[:, 4:8, :].rearrange("p a b -> p (a b)")]
                    for ib in range(NIB):
                        io = i0 + ib * 128
                        if c == NCHUNK - 1:
                            pj = sim_tile((128, C), F32, name="pjt")
                        else:
                            pj = pjregs[ib % 2]
                        nc.tensor.matmul(pj, lhsT=ones_sb,
                                         rhs=bias_sb[0:1, :], start=True,
                                         stop=False, skip_group_check=True)
                        nc.tensor.matmul(pj, lhsT=htsb[:, ib, :], rhs=woutT_sb,
                                         start=False, stop=True,
                                         skip_group_check=True)
                        ot = otp.tile([128, C], F32, tag="out", name="ot")
                        if c == NCHUNK - 1 and ib % 2 == (0 if COPY_FLIP else 1):
                            nc.vector.tensor_copy(ot, pj)
                        else:
                            nc.scalar.copy(ot, pj)
                        last = c == NCHUNK - 1
                        if TAIL_POOL == 1 and last:
                            pool_dma = True
                        elif TAIL_POOL == 2 and last:
                            pool_dma = ib % 2 == 1
                        elif TAIL_POOL == 3:
                            pool_dma = ib % 2 == 1
                        elif TAIL_POOL == 4 and last:
                            pool_dma = ib >= 2
                        elif TAIL_POOL == 5 and last:
                            pool_dma = ib % 2 == 0
                        else:
                            pool_dma = False
                        if pool_dma:
                            nc.gpsimd.dma_start(out=out_d[io:io + 128, :],
                                                in_=ot)
                        else:
                            nc.sync.dma_start(out=out_d[io:io + 128, :], in_=ot)
                return epi

            for c in range(NCHUNK):
                i0 = c * CHUNK
                # 16 accumulators [128, 33] at stride-64 slots over 2 banks
                avt = pav.tile([128, 16, 64], F32, tag="av", name="avt")

                def emit_av(ex, jt, ibs=range(NIB)):
                    for ib in ibs:
                        for h in range(HEADS):
                            idx = ib * HEADS + h
                            exs = ex[:, h * 512 + ib * 128:h * 512 + (ib + 1) * 128]
                            nc.tensor.matmul(
                                avt[:, idx, 0:32],
                                lhsT=exs,
                                rhs=vT_sb[:, jt, h * DH:(h + 1) * DH],
                                start=(jt == 0 and idx % 8 == 0),
                                stop=(jt == NJT - 1),
                                skip_group_check=True,
                            )
                            nc.tensor.matmul(
                                avt[:, idx, 32:33],
                                lhsT=exs,
                                rhs=onesb_sb,
                                start=False, stop=(jt == NJT - 1),
                                skip_group_check=True,
                            )

                # AV emission lags 1 step normally; more at the start of
                # chunks > 0 so the bank WAW (vs the previous epilogue's
                # reads) never parks at the head of the in-order PE queue.
                pending = []
                lag = LAG_START if c > 0 else LAG_C0
                for jt in range(NJT):
                    lo = SPLIT_STEPS.get((c, jt), 0)
                    ex = expp.tile([128, HEADS * 512], BF16, tag="exp", name="ex")
                    sims = []
                    def emit_sims(grp):
                        sim = sim_tile()
                        for hi in range(2):
                            h = grp * 2 + hi
                            nc.tensor.matmul(
                                sim[:, hi * 512:(hi + 1) * 512],
                                lhsT=k_sb[h * DH:(h + 1) * DH,
                                          jt * JT:(jt + 1) * JT],
                                rhs=q_sb[h * DH:(h + 1) * DH, i0:i0 + 512],
                                start=True, stop=True,
                                tile_position=(h * DH, 0),
                            )
                        return sim

                    def emit_exp(grp, sim):
                        exs = ex[:, grp * 1024:(grp + 1) * 1024]
                        if grp == 0:
                            if lo:
                                nc.scalar.activation(
                                    exs[:, 0:lo], sim[:, 0:lo],
                                    mybir.ActivationFunctionType.Exp,
                                    scale=LN2 / 128.0)
                            if lo < 1024:
                                nc.vector.tensor_scalar(
                                    exs[:, lo:1024].bitcast(I16), sim[:, lo:1024],
                                    SCHRAU_B, None, mybir.AluOpType.add)
                        else:
                            nc.scalar.activation(
                                exs, sim, mybir.ActivationFunctionType.Exp,
                                scale=LN2 / 128.0)

                    if AV_SPLIT:
                        sim0 = emit_sims(0)
                        emit_exp(0, sim0)
                        half = None
                        if len(pending) > (2 if jt > 2 else 99):
                            half = pending[0]
                            emit_av(half[0], half[1], ibs=range(0, 2))
                        sim1 = emit_sims(1)
                        emit_exp(1, sim1)
                        if half is not None:
                            emit_av(half[0], half[1], ibs=range(2, 4))
                            pending.pop(0)
                    elif EMIT_ORDER:
                        s0 = emit_sims(0)
                        s1 = emit_sims(1)
                        emit_exp(0, s0)
                        emit_exp(1, s1)
                    else:
                        for grp in range(2):
                            sim = emit_sims(grp)
                            emit_exp(grp, sim)
                    pending.append((ex, jt))
                    cur_lag = (lag if jt < LAG_SW else LAG_MID) if not (c == NCHUNK - 1 and jt > TAIL_JT) else 1
                    if TAPER_ALL:
                        cur_lag = min(cur_lag, max(1, NJT - 1 - jt))
                    while len(pending) > cur_lag:
                        emit_av(*pending.pop(0))
                    if c == 0 and jt in PHASE1B:
                        PHASE1B[jt]()
                while pending:
                    last_ex = pending[0][0]
                    emit_av(*pending.pop(0))
                if dbg and c == 0:
                    nc.sync.dma_start(out=dumps["ex0"], in_=last_ex)
                make_epilogue(avt, c, i0)()

            if dbg:
                nc.sync.dma_start(out=dumps["q"], in_=q_sb[:, :].bitcast(F32))
                nc.sync.dma_start(out=dumps["k"], in_=k_sb[:, :].bitcast(F32))
                nc.sync.dma_start(
                    out=dumps["vT"],
                    in_=vT_sb[:, :, :].rearrange("p a b -> p (a b)"))

    nc.compile()
    return nc


_NC = None


def _get_nc():
    global _NC
    if _NC is None:
        _NC = build_kernel()
    return _NC


def make_in_maps(x, w_qkv, w_out, b_out):
    x = np.ascontiguousarray(np.asarray(x, dtype=np.float32))
    w_qkv = np.asarray(w_qkv, dtype=np.float32)
    w_out = np.asarray(w_out, dtype=np.float32)
    b_out = np.asarray(b_out, dtype=np.float32)

    wqkvT = w_qkv.T.copy()                                # [256, 384]
    wqkvT[:, 0:128] *= SCALE * QPRE                       # fold exp prescale into q
    woutT = w_out.T                                       # [128 hidden, 256]
    blob128 = np.ascontiguousarray(
        np.concatenate([woutT,
                        np.broadcast_to(b_out[None, :], (128, C))], axis=1))

    in_maps = []
    for core in range(8):
        b, qh = divmod(core, 2)
        xb = x[b].reshape(C, NJ)
        # own query half first, then the other half (j-order permuted)
        xp = np.concatenate([xb[:, qh * NI:(qh + 1) * NI],
                             xb[:, (1 - qh) * NI:(2 - qh) * NI]], axis=1)
        blob256 = np.ascontiguousarray(
            np.concatenate([wqkvT, xp], axis=1).astype(ml_dtypes.bfloat16))
        in_maps.append({"blob256": blob256, "blob128": blob128})
    return in_maps


def run_spmd(x, w_qkv, w_out, b_out, **kw):
    nc = _get_nc()
    in_maps = make_in_maps(x, w_qkv, w_out, b_out)
    return run_bass_kernel_spmd(nc, in_maps, core_ids=list(range(8)), **kw)


def assemble(results):
    out = np.empty((4, C, NJ), np.float32)
    for core in range(8):
        b, qh = divmod(core, 2)
        out[b, :, qh * NI:(qh + 1) * NI] = results[core]["out_t"].T
    return out.reshape(4, C, 64, 64)


def kernel(x, w_qkv, w_out, b_out):
    res = run_spmd(x, w_qkv, w_out, b_out)
    return assemble(res.results)



# revision 12
# speedup vs baseline: 1.0001x; 1.0001x over previous
"""Trainium2 Bass kernel for nn_Attention (dense transformer spatial attention).

Reference computation (per batch b of 4):
  X = x[b] reshaped [256, 4096]                      (4096 = 64*64 pixels)
  QKV = w_qkv @ X -> [384, 4096]; q,k,v = split(QKV) each [128, 4096]
  per head h (4 heads x 32 dims): sim = (q_h*scale)^T k_h   [4096, 4096]
  attn = softmax(sim, axis=-1); out_h = attn @ v_h^T        [4096, 32]
  H = concat_heads -> [128, 4096]; out = w_out @ H + b_out  [256, 4096]

Sharding: 8 cores = (batch b in 0..3) x (query half qh in 0..1).
Each core gets full X_b (for K/V) plus its query-half slice, computes
attention output for its 2048 queries over all 4096 keys, and the final
projection.  X columns are permuted per core: [own query half | other half],
so q projections read a contiguous slice and j-order is core-local (softmax
is permutation invariant over keys).

Design notes (tuned against the TimelineSim cost model; steady state is
DVE-exp-paced at ~1.26us/jt-step, wall ~189us):
  - blob256 (weights + permuted x) is BF16: halves input DMA bytes and
    makes every projection a bf16 matmul (always 1 PE cycle/row; an f32r
    moving operand under 256 output columns would run at 4 cycles/row).
    Input pieces are merged (one DMA per column range covering both
    128-row halves via a 3-dim access pattern) to reduce serialization on
    the single-slot HWDGE (~625ns per DMA), and the first piece carries
    only the weights so the first projections start ~1us earlier.
  - sim is computed TRANSPOSED (simT[j,i]) via K=32 row-packed bf16
    matmuls (tile_position=(32h,0)).  THREE rotating [128,1024] PSUM
    buffers hold sim tiles (6 banks); with the 2 AV-accumulator banks
    PSUM is exactly full, which rules out wider exp instructions (2048-col
    tiles would need 4 slots = 10 banks).
  - softmax exp is SPLIT between ScalarE (true exp activation, scale
    folded) and the DVE (Schraudolph approximate exp: q is pre-scaled on
    the host by SCALE*128/ln2, so exp bf16 BITS = int16(sim + B) -- one
    tensor_scalar add with fp32->int16 convert writes bf16-bit-pattern
    output directly; ~3% weight error, cancels further in the softmax
    ratio).  DVE always takes group 0 so each rotating sim buffer
    alternates consumer engines.  SPLIT_STEPS hands ScalarE most of the
    DVE tile on the first step of chunks 1-3 so the DVE can absorb the
    previous chunk's epilogue burst (reciprocal/normalize/copies) without
    idling.  Donating columns on OTHER steps always loses: a sub-tile
    donation adds a ~185ns ScalarE instruction overhead and ties both
    engines to one PSUM tile, stalling the 3-slot rotation.
  - vT is projected DIRECTLY as x^T @ wvT (lhsT = the resident x tile,
    K=256): no v tensor, no PE transposes, no extra evacuation -- saves
    ~4096 vector-engine columns vs the transpose pipeline and avoids the
    DMA-xbar transpose races.  The softmax denominators come from an N=1
    ones-column matmul per AV accumulator (~1 PE cycle each).
  - AV is FLIPPED vs the naive layout: stationary = exp tile [128j x
    128i], moving = vT [128j, 32].  16 accumulators [128,33] at stride-64
    slots share 2 PSUM banks; only the first matmul touching each bank
    uses start=True.  AV emission lags the exp stream (LAG_START after a
    chunk boundary, LAG_MID mid-chunk, 1 for the last steps of the final
    chunk) so accumulator-bank WAW never parks at the head of the
    in-order PE queue; the lag values were swept against TimelineSim.
  - The flipped AV emits h as [i, hd]; per 128-query block it is
    normalized (reciprocal of the ones column, free-dim broadcast
    multiply), PE-transposed back to [hd, i] (f32r) and projected with
    full-width woutT in one N=256 f32r matmul.  The bias is pre-loaded
    into the projection PSUM by a K=1 ones-row matmul so the final
    evacuation is a plain copy on whichever engine is idler.  Epilogue
    transposes/projections reuse consumed avt regions; the last chunk
    routes projections through the then-idle sim buffers instead.
  - Phase 1 evacuations alternate ScalarE/DVE under the input-DMA shadow
    (q first-piece on DVE so k and q evacuate in parallel); identity
    transposes pre-warm the PE p-state; the second x-half's K and vT
    production is interleaved into chunk 0 of the main loop through the
    same rotating sim buffers.
  Measured dead ends (cost model): fp8 q/k with DoubleRow matmuls halves
  sim PE time but adds ~4.5%% output error (gate 2%%); fp8 exp tiles
  either NaN on >5.5-sigma sims (e4m3) or add ~2.5%% noise (e5m2); GpSimd
  cannot touch PSUM; DMA cannot read PSUM; DVE 2x/4x modes need all-SBUF
  or 2-byte inputs, and sim is forced f32-in-PSUM.
"""

import math
import os

import ml_dtypes
import numpy as np

def _k(name, default):
    return int(os.environ.get(name, default))

import concourse.bacc as bacc
import concourse.bass as bass
import concourse.masks as masks
import concourse.mybir as mybir
import concourse.tile as tile
from concourse.bass_utils import run_bass_kernel_spmd

F32 = mybir.dt.float32
F32R = mybir.dt.float32r
BF16 = mybir.dt.bfloat16
I16 = mybir.dt.int16

HEADS = 4
DH = 32                      # dim per head
C = 256                      # input channels
NJ = 4096                    # keys per batch (64*64)
NI = 2048                    # queries per core (half of 4096)
JT = 128                     # j tile (partition dim of simT)
NJT = NJ // JT               # 32 j tiles
CHUNK = 512                  # i chunk held in AV psum accumulators
NCHUNK = NI // CHUNK         # 4
NIB = CHUNK // 128           # 4 i-blocks per chunk
SCALE = float(DH) ** -0.5
LN2 = math.log(2.0)
# q is pre-scaled by SCALE * 128/ln2 on the host; ScalarE exp then uses
# scale=ln2/128, and the DVE Schraudolph path just adds SCHRAU_B and
# converts to int16 (the bf16 bit pattern of exp).
QPRE = 128.0 / LN2
SCHRAU_C = 0.05              # Schraudolph correction (centers rel err ~+-3%)
SCHRAU_B = 128.0 * (127.0 - SCHRAU_C) + 0.5   # +0.5: f32->i16 truncates

XW = 384 + NJ                # blob256 width: [wq|wk|wvT (384) | x perm (4096)]

# (chunk, jt) -> columns of the DVE's group-0 tile that ScalarE takes
# instead (a partial hole: smooth rebalancing of ScalarE ~ DVE busy time,
# and air for the DVE's epilogue burst at chunk starts).
SPLIT_COLS = _k("SPLIT_COLS", 960)
SPLIT_MODE = _k("SPLIT_MODE", 0)
if SPLIT_MODE == 0:
    SPLIT_STEPS = {(c, 0): SPLIT_COLS for c in (1, 2, 3)}
elif SPLIT_MODE == 1:
    SPLIT_STEPS = {(c, 1): 1024 for c in (1, 2, 3)}
elif SPLIT_MODE == 2:
    SPLIT_STEPS = {(c, 0): 1024 for c in (1, 2, 3)}
elif SPLIT_MODE == 3:
    SPLIT_STEPS = {(c, 0): 960 for c in (1, 2, 3)}
    SPLIT_STEPS.update({(c, 2): 512 for c in (1, 2, 3)})
elif SPLIT_MODE == 4:
    SPLIT_STEPS = {(c, 0): 960 for c in (1, 2, 3)}
    SPLIT_STEPS[(0, 1)] = 512
elif SPLIT_MODE == 5:
    SPLIT_STEPS = {}
_SE = _k("SPLIT_EVERY", 0)
if _SE:
    for _c in range(4):
        for _jt in range(4, 32, 2):
            SPLIT_STEPS.setdefault((_c, _jt), _SE)
LAG_START = _k("LAG_START", 7)
LAG_C0 = _k("LAG_C0", 2)
LAG_MID = _k("LAG_MID", 2)
LAG_SW = _k("LAG_SW", 12)
TAIL_JT = _k("TAIL_JT", 29)
TAPER_ALL = _k("TAPER_ALL", 0)
FINE_PRO = _k("FINE_PRO", 2)
MERGE_DMA = _k("MERGE_DMA", 1)
EMIT_ORDER = _k("EMIT_ORDER", 0)
AV_SPLIT = _k("AV_SPLIT", 0)
SPLIT_EVERY = _k("SPLIT_EVERY", 0)
P1B_MODE = _k("P1B_MODE", 0)
TAIL_POOL = _k("TAIL_POOL", 5)
PRO_POOL = _k("PRO_POOL", 0)
WARMUP_N = _k("WARMUP_N", 10)


def build_kernel(dbg=False):
    nc = bacc.Bacc("TRN2", debug=False, num_devices=8)

    blob256_d = nc.dram_tensor("blob256", [C, XW], BF16, kind="ExternalInput").ap()
    blob128_d = nc.dram_tensor("blob128", [128, 2 * C], F32R, kind="ExternalInput").ap()
    out_d = nc.dram_tensor("out_t", [NI, C], F32, kind="ExternalOutput").ap()
    if dbg:
        dumps = {n: nc.dram_tensor("dump_" + n, s, d, kind="ExternalOutput").ap()
                 for n, s, d in [
                     ("q", [128, NI], F32), ("k", [128, NJ], F32),
                     ("vT", [128, NJT * 128], BF16),
                     ("rec", [128, NCHUNK * 16], F32),
                     ("ex0", [128, 2048], BF16)]}

    with tile.TileContext(nc) as tc:
        with (
            tc.tile_pool(name="singles", bufs=1) as singles,
            tc.tile_pool(name="expp", bufs=_k("EXPP", 32)) as expp,
            tc.tile_pool(name="hp", bufs=_k("HP", 3)) as hp,
            tc.tile_pool(name="htp", bufs=_k("HTP", 3)) as htp,
            tc.tile_pool(name="otp", bufs=_k("OTP", 4)) as otp,
            tc.tile_pool(name="recp", bufs=_k("RECP", 2)) as recp,
            tc.tile_pool(name="psim", bufs=1, space="PSUM") as psim,
            tc.tile_pool(name="pav", bufs=1, space="PSUM") as pav,
        ):
            # ---- resident SBUF tensors ----
            blob_sb = singles.tile([128, 2, XW], BF16)
            wq_sb = blob_sb[:, :, 0:128]
            wk_sb = blob_sb[:, :, 128:256]
            wvT_sb = blob_sb[:, :, 256:384]
            x_sb = blob_sb[:, :, 384:XW]
            b128_sb = singles.tile([128, 2 * C], F32R)
            woutT_sb = b128_sb[:, 0:C]
            bias_sb = b128_sb[:, C:2 * C]
            q_sb = singles.tile([128, NI], F32R)      # rows = 4h x 32d (prescaled)
            k_sb = singles.tile([128, NJ], F32R)
            # vT[j, jt, hd]: projected directly as xT @ wvT (no transpose
            # pipeline); softmax denominators come from a separate ones
            # column via N=1 matmuls
            vT_sb = singles.tile([128, NJT, 128], BF16)
            onesb_sb = singles.tile([128, 1], BF16)
            idr_sb = singles.tile([128, 128], F32R)   # identity for f32r transposes
            ones_sb = singles.tile([1, 128], F32R)    # K=1 bias-broadcast lhsT

            # rotating sim-chain PSUM slot allocator (3 tags x [128,1024])
            SIMTAGS = ("simA", "simB", "simC")
            sim_ctr = [0]

            def sim_tile(shape=(128, 1024), dtype=F32, name="sim"):
                tag = SIMTAGS[sim_ctr[0] % 3]
                sim_ctr[0] += 1
                return psim.tile(list(shape), dtype, tag=tag, name=name)

            # ---- input DMAs (SP engine); x own-query-half first.  The first
            # transfer carries the weights AND the first 512 x columns in one
            # piece (fewer serial DMA latencies before the q projection).
            if MERGE_DMA:
                blob_src = blob256_d.rearrange("(ct p) c -> p ct c", ct=2)
                if FINE_PRO == 3:
                    pieces = ((0, 256), (384, 512), (256, 128), (896, 512),
                              (1408, 512), (1920, 512), (2432, 1024),
                              (3456, 1024))
                elif FINE_PRO == 2:
                    pieces = ((0, 256), (256, 640), (896, 512), (1408, 512),
                              (1920, 512), (2432, 1024), (3456, 1024))
                elif FINE_PRO:
                    pieces = ((0, 384), (384, 512), (896, 512), (1408, 1024),
                              (2432, 2048))
                else:
                    pieces = ((0, 896), (896, 512), (1408, 1024), (2432, 2048))
                for n, (lo, w) in enumerate(pieces):
                    eng = nc.gpsimd if (PRO_POOL and n < 2) else nc.sync
                    eng.dma_start(out=blob_sb[:, :, lo:lo + w],
                                  in_=blob_src[:, :, lo:lo + w])
            else:
                for lo, w in ((0, 896), (896, 512), (1408, 1024), (2432, 2048)):
                    for ct in range(2):
                        nc.sync.dma_start(out=blob_sb[:, ct, lo:lo + w],
                                          in_=blob256_d[ct * 128:(ct + 1) * 128,
                                                        lo:lo + w])
            nc.sync.dma_start(out=b128_sb, in_=blob128_d)

            # identity built once in plain f32 on GpSimd (idle engine), then
            # DVE-converted to the f32r copy the transposes need
            idf_sb = singles.tile([128, 128], F32)
            masks.make_identity(nc, idf_sb)
            nc.vector.tensor_copy(idr_sb, idf_sb)
            nc.vector.memset(onesb_sb, 1.0)
            nc.vector.memset(ones_sb.bitcast(F32), 1.0)

            # trigger the ScalarE exp table load (~2.7us) during phase 1
            warm = singles.tile([1, 1], F32)
            nc.vector.memset(warm, 0.0)
            nc.scalar.activation(warm, warm, mybir.ActivationFunctionType.Exp)

            # warm the PE p-state before the inputs arrive: identity
            # transposes keep the array busy through the cold ramp so the
            # first real projections run at full clock
            pwu = pav.tile([128, 128], F32, tag="av", name="pwu")
            for _ in range(WARMUP_N):
                nc.tensor.transpose(pwu, idf_sb, idf_sb, )

            # ---- phase 1a: q, k/v half 0, vT half 0 ----
            def project(w_slice, x_lo, width, name):
                """[128, width] psum tile = w_slice.T @ x[:, x_lo:x_lo+width]."""
                ps = sim_tile((128, width), F32, name=name)
                for nt0 in range(0, width, 512):
                    w_seg = min(512, width - nt0)
                    for ct in range(2):
                        nc.tensor.matmul(
                            ps[:, nt0:nt0 + w_seg],
                            lhsT=w_slice[:, ct, :],
                            rhs=x_sb[:, ct, x_lo + nt0:x_lo + nt0 + w_seg],
                            start=(ct == 0), stop=(ct == 1),
                        )
                return ps

            # fine-grained first evacs: the first sims need only k cols 0:512
            # and q cols 0:512, so those 512-wide pieces come first and the
            # exp pipeline starts ~2us earlier
            def emit_p1(w_slice, dst, lo, w, dve):
                ps = project(w_slice, lo, w, "ps_p1")
                if dve:
                    nc.vector.tensor_copy(dst[:, lo:lo + w], ps)
                else:
                    nc.scalar.copy(dst[:, lo:lo + w], ps)

            if P1B_MODE == 3:
                p1_list = [
                    (wk_sb, k_sb, 0, 256, False), (wq_sb, q_sb, 0, 512, True),
                    (wk_sb, k_sb, 256, 256, False)]
            elif FINE_PRO:
                p1_list = [
                    (wk_sb, k_sb, 0, 256, False), (wq_sb, q_sb, 0, 512, True),
                    (wk_sb, k_sb, 256, 256, False),
                    (wk_sb, k_sb, 512, 512, False),
                    (wq_sb, q_sb, 512, 512, False),
                    (wq_sb, q_sb, 1024, 1024, False),
                    (wk_sb, k_sb, 1024, 1024, False)]
            else:
                p1_list = [
                    (wk_sb, k_sb, 0, 512, False), (wq_sb, q_sb, 0, 512, False),
                    (wk_sb, k_sb, 512, 512, False), (wq_sb, q_sb, 512, 512, False),
                    (wq_sb, q_sb, 1024, 1024, False),
                    (wk_sb, k_sb, 1024, 1024, False)]
            for w_slice, dst, lo, w, dve in p1_list:
                emit_p1(w_slice, dst, lo, w, dve)

            def emit_vtn(jt0, n, via_sim, dve):
                """vT tiles jt0..jt0+n-1 projected directly from x:
                vt[j, hd] = x[:, j]^T @ wvT (lhsT = x slice, K = 256)."""
                if via_sim:
                    tp = sim_tile((128, n, 128), F32, name="vtp")
                else:
                    tp = pav.tile([128, n, 128], F32, tag="av", name="vtp",
                                  padded_shape=[128, 4, 128])
                for i4 in range(n):
                    jt = jt0 + i4
                    for ct in range(2):
                        nc.tensor.matmul(
                            tp[:, i4, :],
                            lhsT=x_sb[:, ct, jt * 128:(jt + 1) * 128],
                            rhs=wvT_sb[:, ct, :],
                            start=(ct == 0), stop=(ct == 1),
                        )
                if dve:
                    nc.vector.tensor_copy(vT_sb[:, jt0:jt0 + n, :], tp)
                else:
                    nc.scalar.copy(vT_sb[:, jt0:jt0 + n, :], tp)

            def emit_vt4(g, via_sim):
                emit_vtn(4 * g, 4, via_sim, g % 2 == 0)

            for g in range(1 if P1B_MODE == 3 or VT_SIM == 2 else 4):
                emit_vt4(g, via_sim=(VT_SIM == 1))         # vT half 0

            # ---- phase 1b pieces, interleaved into chunk 0 of the main loop,
            # rotating through the same sim-chain psum slots.  Evacuations
            # alternate ScalarE (k) / DVE (vT) to spread the load.
            def emit_k1_piece(n):
                lo = 2048 + n * 1024
                ps = project(wk_sb, lo, 1024, "k1p")
                nc.scalar.copy(k_sb[:, lo:lo + 1024], ps)

            def emit_piece(w_slice, dst, lo, w, dve=False):
                ps = project(w_slice, lo, w, "p1b")
                if dve:
                    nc.vector.tensor_copy(dst[:, lo:lo + w], ps)
                else:
                    nc.scalar.copy(dst[:, lo:lo + w], ps)

            if P1B_MODE == 0:
                PHASE1B = {
                    2: lambda: emit_k1_piece(0),
                    5: lambda: emit_k1_piece(1),
                    8: lambda: emit_vtn(16, 4, True, True),
                    10: lambda: emit_vtn(20, 4, True, False),
                    12: lambda: emit_vtn(24, 4, True, True),
                    14: lambda: emit_vtn(28, 4, True, False),
                }
                if VT_SIM == 2:
                    PHASE1B[1] = lambda: emit_vt4(1, via_sim=True)
                    PHASE1B[3] = lambda: emit_vt4(2, via_sim=True)
                    PHASE1B[6] = lambda: emit_vt4(3, via_sim=True)
            elif P1B_MODE == 1:
                PHASE1B = {
                    6: lambda: emit_k1_piece(0),
                    10: lambda: emit_vtn(16, 4, True, True),
                    13: lambda: emit_k1_piece(1),
                    16: lambda: emit_vtn(20, 4, True, False),
                    20: lambda: emit_vtn(24, 4, True, True),
                    24: lambda: emit_vtn(28, 4, True, False),
                }
            elif P1B_MODE == 2:
                PHASE1B = {
                    4: lambda: emit_k1_piece(0),
                    8: lambda: emit_vtn(16, 4, True, True),
                    10: lambda: emit_k1_piece(1),
                    14: lambda: emit_vtn(20, 4, True, False),
                    18: lambda: emit_vtn(24, 4, True, True),
                    22: lambda: emit_vtn(28, 4, True, False),
                }
            else:
                # need-ordered: each piece lands just before its first use,
                # so early ScalarE/DVE priority goes to the exp stream
                PHASE1B = {
                    1: lambda: emit_piece(wk_sb, k_sb, 512, 512),
                    2: lambda: emit_vt4(1, via_sim=True),
                    3: lambda: emit_piece(wk_sb, k_sb, 1024, 1024),
                    5: lambda: emit_vt4(2, via_sim=True),
                    6: lambda: emit_piece(wq_sb, q_sb, 512, 512),
                    8: lambda: emit_k1_piece(0),
                    9: lambda: emit_vt4(3, via_sim=True),
                    10: lambda: emit_vtn(16, 4, True, True),
                    12: lambda: emit_k1_piece(1),
                    14: lambda: emit_vtn(20, 4, True, False),
                    16: lambda: emit_piece(wq_sb, q_sb, 1024, 1024),
                    18: lambda: emit_vtn(24, 4, True, True),
                    22: lambda: emit_vtn(28, 4, True, False),
                }

            # ---- phase 2: attention main loop ----
            # Each chunk's epilogue is DEFERRED into the next chunk (flushed
            # after its first two sim/exp steps) so the exp engines never
            # starve behind epilogue PE work at chunk boundaries.
            deferred_epi = [None]

            def make_epilogue(avt, c, i0):
                def epi():
                    import contextlib
                    dl = (tc.high_priority(-EPI_DELAY)
                          if EPI_DELAY and c < NCHUNK - 1
                          else contextlib.nullcontext())
                    with dl:
                        _epi_body()

                def _epi_body():
                    rec = recp.tile([128, 16, 1], F32, tag="rec", name="rec")
                    if FINAL2 and c == NCHUNK - 1:
                        # split normalize in halves so the first transposes
                        # start ~0.4us earlier in the tail
                        hsb2 = hp.tile([128, NIB * HEADS, DH], F32R, tag="h",
                                       name="hsb")
                        for hf in range(2):
                            s8 = slice(hf * 8, hf * 8 + 8)
                            nc.vector.reciprocal(out=rec[:, s8, 0],
                                                 in_=avt[:, s8, 32])
                            nc.vector.tensor_tensor(
                                out=hsb2[:, s8, :],
                                in0=avt[:, s8, 0:32],
                                in1=rec[:, s8, :].to_broadcast((128, 8, DH)),
                                op=mybir.AluOpType.mult,
                            )
                        _finish(hsb2)
                        return
                    nc.vector.reciprocal(out=rec[:, :, 0], in_=avt[:, :, 32])
                    if dbg:
                        nc.sync.dma_start(
                            out=dumps["rec"][:, c * 16:(c + 1) * 16],
                            in_=rec[:, :, 0])
                    # one fused normalize for all 16 accumulators (4 separate
                    # TTs pay ~160ns per-op overhead each)
                    hsb = hp.tile([128, NIB * HEADS, DH], F32R, tag="h",
                                  name="hsb")
                    nc.vector.tensor_tensor(
                        out=hsb,
                        in0=avt[:, :, 0:32],
                        in1=rec.to_broadcast((128, NIB * HEADS, DH)),
                        op=mybir.AluOpType.mult,
                    )
                    _finish(hsb)

                def _finish(hsb):
                    hsbs = [hsb[:, ib * HEADS:(ib + 1) * HEADS, :]
                            .rearrange("p h d -> p (h d)") for ib in range(NIB)]
                    # all 4 transposes into bank B (accums 8..15, 512B each)
                    tps = avt[:, 8:16, :].rearrange("p a b -> p (a b)").bitcast(F32R)
                    for ib in range(NIB):
                        nc.tensor.transpose(tps[:, ib * 128:(ib + 1) * 128],
                                            hsbs[ib], idr_sb)
                    htsb = htp.tile([128, 4, 128], F32R, tag="ht", name="htsb")
                    nc.scalar.copy(htsb[:, 0:2, :],
                                   tps[:, 0:256].bitcast(F32).bitcast(F32R))
                    nc.vector.tensor_copy(htsb[:, 2:4, :],
                                          tps[:, 256:512].bitcast(F32))
                    # projections ping-pong through bank A (accums 0..7).
                    # In the final chunk the sim slots are idle and have no
                    # false whole-tile WAR against avt, so projections go there
                    # and run fully parallel instead of serializing behind each
                    # ot read.
                    pjregs = [avt[:, 0:4, :].rearrange("p a b -> p (a b)"),
                              avt[:, 4:8, :].rearrange("p a b -> p (a b)")]
                    for ib in range(NIB):
                        io = i0 + ib * 128
                        if c == NCHUNK - 1:
                            pj = sim_tile((128, C), F32, name="pjt")
                        else:
                            pj = pjregs[ib % 2]
                        nc.tensor.matmul(pj, lhsT=ones_sb,
                                         rhs=bias_sb[0:1, :], start=True,
                                         stop=False, skip_group_check=True)
                        nc.tensor.matmul(pj, lhsT=htsb[:, ib, :], rhs=woutT_sb,
                                         start=False, stop=True,
                                         skip_group_check=True)
                        ot = otp.tile([128, C], F32, tag="out", name="ot")
                        if c == NCHUNK - 1 and ib % 2 == (0 if COPY_FLIP else 1):
                            nc.vector.tensor_copy(ot, pj)
                        else:
                            nc.scalar.copy(ot, pj)
                        last = c == NCHUNK - 1
                        if TAIL_POOL == 1 and last:
                            pool_dma = True
                        elif TAIL_POOL == 2 and last:
                            pool_dma = ib % 2 == 1
                        elif TAIL_POOL == 3:
                            pool_dma = ib % 2 == 1
                        elif TAIL_POOL == 4 and last:
                            pool_dma = ib >= 2
                        elif TAIL_POOL == 5 and last:
                            pool_dma = ib % 2 == 0
                        else:
                            pool_dma = False
                        if pool_dma:
                            nc.gpsimd.dma_start(out=out_d[io:io + 128, :],
                                                in_=ot)
                        else:
                            nc.sync.dma_start(out=out_d[io:io + 128, :], in_=ot)
                return epi

            for c in range(NCHUNK):
                i0 = c * CHUNK
                # 16 accumulators [128, 33] at stride-64 slots over 2 banks
                avt = pav.tile([128, 16, 64], F32, tag="av", name="avt")

                def emit_av(ex, jt, ibs=range(NIB)):
                    for ib in ibs:
                        for h in range(HEADS):
                            idx = ib * HEADS + h
                            exs = ex[:, h * 512 + ib * 128:h * 512 + (ib + 1) * 128]
                            nc.tensor.matmul(
                                avt[:, idx, 0:32],
                                lhsT=exs,
                                rhs=vT_sb[:, jt, h * DH:(h + 1) * DH],
                                start=(jt == 0 and idx % 8 == 0),
                                stop=(jt == NJT - 1),
                                skip_group_check=True,
                            )
                            nc.tensor.matmul(
                                avt[:, idx, 32:33],
                                lhsT=exs,
                                rhs=onesb_sb,
                                start=False, stop=(jt == NJT - 1),
                                skip_group_check=True,
                            )

                # AV emission lags 1 step normally; more at the start of
                # chunks > 0 so the bank WAW (vs the previous epilogue's
                # reads) never parks at the head of the in-order PE queue.
                pending = []
                lag = LAG_START if c > 0 else LAG_C0
                for jt in range(NJT):
                    lo = SPLIT_STEPS.get((c, jt), 0)
                    ex = expp.tile([128, HEADS * 512], BF16, tag="exp", name="ex")
                    sims = []
                    def emit_sims(grp):
                        sim = sim_tile()
                        for hi in range(2):
                            h = grp * 2 + hi
                            nc.tensor.matmul(
                                sim[:, hi * 512:(hi + 1) * 512],
                                lhsT=k_sb[h * DH:(h + 1) * DH,
                                          jt * JT:(jt + 1) * JT],
                                rhs=q_sb[h * DH:(h + 1) * DH, i0:i0 + 512],
                                start=True, stop=True,
                                tile_position=(h * DH, 0),
                            )
                        return sim

                    def emit_exp(grp, sim):
                        exs = ex[:, grp * 1024:(grp + 1) * 1024]
                        if grp == (1 if GRP_SWAP else 0):
                            if lo:
                                nc.scalar.activation(
                                    exs[:, 0:lo], sim[:, 0:lo],
                                    mybir.ActivationFunctionType.Exp,
                                    scale=LN2 / 128.0)
                            if lo < 1024:
                                nc.vector.tensor_scalar(
                                    exs[:, lo:1024].bitcast(I16), sim[:, lo:1024],
                                    SCHRAU_B, None, mybir.AluOpType.add)
                        else:
                            nc.scalar.activation(
                                exs, sim, mybir.ActivationFunctionType.Exp,
                                scale=LN2 / 128.0)

                    if AV_SPLIT:
                        sim0 = emit_sims(0)
                        emit_exp(0, sim0)
                        half = None
                        if len(pending) > (2 if jt > 2 else 99):
                            half = pending[0]
                            emit_av(half[0], half[1], ibs=range(0, 2))
                        sim1 = emit_sims(1)
                        emit_exp(1, sim1)
                        if half is not None:
                            emit_av(half[0], half[1], ibs=range(2, 4))
                            pending.pop(0)
                    elif EMIT_ORDER:
                        s0 = emit_sims(0)
                        s1 = emit_sims(1)
                        emit_exp(0, s0)
                        emit_exp(1, s1)
                    else:
                        for grp in range(2):
                            sim = emit_sims(grp)
                            emit_exp(grp, sim)
                    pending.append((ex, jt))
                    cur_lag = (lag if jt < LAG_SW else LAG_MID) if not (c == NCHUNK - 1 and jt > TAIL_JT) else 1
                    if TAPER_ALL:
                        cur_lag = min(cur_lag, max(1, NJT - 1 - jt))
                    while len(pending) > cur_lag:
                        emit_av(*pending.pop(0))
                    if c == 0 and jt in PHASE1B:
                        PHASE1B[jt]()
                while pending:
                    last_ex = pending[0][0]
                    emit_av(*pending.pop(0))
                if dbg and c == 0:
                    nc.sync.dma_start(out=dumps["ex0"], in_=last_ex)
                make_epilogue(avt, c, i0)()

            if dbg:
                nc.sync.dma_start(out=dumps["q"], in_=q_sb[:, :].bitcast(F32))
                nc.sync.dma_start(out=dumps["k"], in_=k_sb[:, :].bitcast(F32))
                nc.sync.dma_start(
                    out=dumps["vT"],
                    in_=vT_sb[:, :, :].rearrange("p a b -> p (a b)"))

    nc.compile()
    return nc


_NC = None


def _get_nc():
    global _NC
    if _NC is None:
        _NC = build_kernel()
    return _NC


def make_in_maps(x, w_qkv, w_out, b_out):
    x = np.ascontiguousarray(np.asarray(x, dtype=np.float32))
    w_qkv = np.asarray(w_qkv, dtype=np.float32)
    w_out = np.asarray(w_out, dtype=np.float32)
    b_out = np.asarray(b_out, dtype=np.float32)

    wqkvT = w_qkv.T.copy()                                # [256, 384]
    wqkvT[:, 0:128] *= SCALE * QPRE                       # fold exp prescale into q
    woutT = w_out.T                                       # [128 hidden, 256]
    blob128 = np.ascontiguousarray(
        np.concatenate([woutT,
                        np.broadcast_to(b_out[None, :], (128, C))], axis=1))

    in_maps = []
    for core in range(8):
        b, qh = divmod(core, 2)
        xb = x[b].reshape(C, NJ)
        # own query half first, then the other half (j-order permuted)
        xp = np.concatenate([xb[:, qh * NI:(qh + 1) * NI],
                             xb[:, (1 - qh) * NI:(2 - qh) * NI]], axis=1)
        blob256 = np.ascontiguousarray(
            np.concatenate([wqkvT, xp], axis=1).astype(ml_dtypes.bfloat16))
        in_maps.append({"blob256": blob256, "blob128": blob128})
    return in_maps


def run_spmd(x, w_qkv, w_out, b_out, **kw):
    nc = _get_nc()
    in_maps = make_in_maps(x, w_qkv, w_out, b_out)
    return run_bass_kernel_spmd(nc, in_maps, core_ids=list(range(8)), **kw)


def assemble(results):
    out = np.empty((4, C, NJ), np.float32)
    for core in range(8):
        b, qh = divmod(core, 2)
        out[b, :, qh * NI:(qh + 1) * NI] = results[core]["out_t"].T
    return out.reshape(4, C, 64, 64)


def kernel(x, w_qkv, w_out, b_out):
    res = run_spmd(x, w_qkv, w_out, b_out)
    return assemble(res.results)

